# revision 31
# baseline (speedup 1.0000x reference)
"""Distributed RoPE causal attention for Trainium2 (8 NeuronCores).

Problem: B=2, L=2048, H=16 heads, D=64; y = Attn(x) with RoPE on q,k,
causal softmax, then output projection. fp32 I/O.

Sharding: each core owns 2 heads (tensor-parallel over the 16 heads) for
BOTH batches; after attention, a single 8-core AllToAll redistributes the
per-head outputs so each core holds the full hidden vector for one
(batch, L/4-block) slice, on which it runs the (replicated-weight) output
projection. Matmuls run in bf16 (fp32 PSUM accumulation); RoPE/softmax
plumbing in fp32.
"""
import sys

sys.path.insert(0, "/opt/trn_rl_repo")

import numpy as np
import ml_dtypes

import concourse.bass as bass
import concourse.tile as tile
from concourse import bacc, mybir
from concourse import bass_utils

B, L, H, D = 2, 2048, 16, 64
HID = H * D
NC = 8
CH = 512          # lq chunk width
NCH = L // CH     # 4 chunks per batch
NT = L // 128     # 16 k-tiles of 128 per batch
F32 = mybir.dt.float32
BF16 = mybir.dt.bfloat16
AF = mybir.ActivationFunctionType
ALU = mybir.AluOpType

_CACHE = {}


def build():
    nc = bacc.Bacc("TRN2", target_bir_lowering=False, debug=False, num_devices=NC)

    xT_e = nc.dram_tensor("xT", [B, HID, L], BF16, kind="ExternalInput")
    wq_e = nc.dram_tensor("wq", [HID, 128], BF16, kind="ExternalInput")
    wk_e = nc.dram_tensor("wk", [HID, 128], BF16, kind="ExternalInput")
    wv_e = nc.dram_tensor("wv", [HID, 128], BF16, kind="ExternalInput")
    wo_e = nc.dram_tensor("wo", [HID, HID], BF16, kind="ExternalInput")
    cos_e = nc.dram_tensor("cos2", [128, L], F32, kind="ExternalInput")
    sin_e = nc.dram_tensor("sinpm", [128, L], F32, kind="ExternalInput")
    tri_e = nc.dram_tensor("tri", [128, 128], BF16, kind="ExternalInput")
    out_e = nc.dram_tensor("out", [CH, HID], F32, kind="ExternalOutput")

    xT_r = xT_e.ap().rearrange("b (kt p) l -> b p kt l", p=128)

    with tile.TileContext(nc) as tc:
        with tc.tile_pool(name="const", bufs=1) as cpool, \
             tc.tile_pool(name="persist", bufs=1) as ppool, \
             tc.tile_pool(name="xin", bufs=3) as xpool, \
             tc.tile_pool(name="tmp", bufs=3) as tpool, \
             tc.tile_pool(name="ptp", bufs=6) as ptpool, \
             tc.tile_pool(name="osb", bufs=2) as opool, \
             tc.tile_pool(name="fin", bufs=2) as fpool, \
             tc.tile_pool(name="ps_proj", bufs=3, space="PSUM") as ps_proj, \
             tc.tile_pool(name="ps_s", bufs=2, space="PSUM") as ps_s, \
             tc.tile_pool(name="ps_o", bufs=3, space="PSUM") as ps_o, \
             tc.tile_pool(name="dram", bufs=1, space="DRAM") as dpool:

            wq_sb = cpool.tile([128, 8, 128], BF16)
            wk_sb = cpool.tile([128, 8, 128], BF16)
            wv_sb = cpool.tile([128, 8, 128], BF16)
            wo_sb = cpool.tile([128, 8, HID], BF16)
            nc.sync.dma_start(wq_sb[:], wq_e.ap().rearrange("(kt p) m -> p kt m", p=128))
            nc.sync.dma_start(wk_sb[:], wk_e.ap().rearrange("(kt p) m -> p kt m", p=128))
            nc.sync.dma_start(wv_sb[:], wv_e.ap().rearrange("(kt p) m -> p kt m", p=128))
            cos_sb = cpool.tile([128, L], F32)
            sin_sb = cpool.tile([128, L], F32)
            tri2_sb = cpool.tile([128, 2, 128], BF16)
            nc.sync.dma_start(tri2_sb[:, 0], tri_e[:, :])
            nc.sync.dma_start(tri2_sb[:, 1], tri_e[:, :])

            # persistent per-core tensors
            qT_sb = ppool.tile([128, B, L], BF16)   # [64*hl+d, b, l]
            kT_sb = ppool.tile([128, B, L], BF16)
            v_sb = ppool.tile([128, B, NT, 130], BF16)  # [lk%128, b, lk//128, 65*hl + (d|ones)]

            # one AllGather per lq-block (both batches), pipelined behind compute;
            # each core later picks its own (batch, block) out of ag_all by core id
            ag_all = dpool.tile([NCH, NC * 256, CH], BF16)
            ag_in = [dpool.tile([256, CH], BF16, name=f"agin{i}", tag=f"agin{i}")
                     for i in range(NCH)]

            for j in range(NCH):
                for b in range(B):
                    if j == 1 and b == 0:
                        # late-load the (big) output-projection weight: only needed
                        # after the AllGathers, keeps startup DMA on the critical inputs
                        nc.sync.dma_start(
                            wo_sb[:], wo_e.ap().rearrange("(kt p) m -> p kt m", p=128))
                    ls = j * CH
                    xc = xpool.tile([128, 8, CH], BF16, name="xc")
                    nc.sync.dma_start(xc[:], xT_r[b][:, :, ls:ls + CH])
                    if b == 0:
                        nc.sync.dma_start(cos_sb[:, ls:ls + CH], cos_e[:, ls:ls + CH])
                        nc.sync.dma_start(sin_sb[:, ls:ls + CH], sin_e[:, ls:ls + CH])

                    # ---- Q, K projections + RoPE ----
                    for w_sb, dst in ((wq_sb, qT_sb), (wk_sb, kT_sb)):
                        pp = ps_proj.tile([128, CH], F32, tag="proj", name="pp")
                        for kt in range(8):
                            nc.tensor.matmul(pp[:], w_sb[:, kt], xc[:, kt],
                                             start=(kt == 0), stop=(kt == 7))
                        t1 = tpool.tile([128, CH], F32, name="t1")
                        nc.vector.tensor_mul(t1[:], pp[:], cos_sb[:, ls:ls + CH])
                        t2 = tpool.tile([128, CH], F32, name="t2")
                        for seg in range(4):
                            po, pi = 32 * seg, 32 * (seg ^ 1)
                            nc.vector.tensor_mul(t2[po:po + 32, :], pp[pi:pi + 32, :],
                                                 sin_sb[po:po + 32, ls:ls + CH])
                        nc.vector.tensor_add(dst[:, b, ls:ls + CH], t1[:], t2[:])

                    # ---- V projection (+ ones column for denominators) ----
                    for tl in range(4):
                        t = 4 * j + tl
                        pv = ps_proj.tile([128, 128], F32, tag="proj", name="pv")
                        for kt in range(8):
                            nc.tensor.matmul(pv[:], xc[:, kt, 128 * tl:128 * tl + 128],
                                             wv_sb[:, kt], start=(kt == 0), stop=(kt == 7))
                        nc.vector.memset(v_sb[:, b, t, 64::65], 1.0)
                        vv = v_sb.rearrange("p b t (hl e) -> p b t hl e", e=65)
                        nc.vector.tensor_copy(
                            vv[:, b, t, :, 0:64],
                            pv.rearrange("p (hl d) -> p hl d", d=64),
                        )

                    # ---- attention for lq chunk j, both local heads ----
                    po_t = [ps_o.tile([65, CH], F32, tag="o", name=f"po{hl}")
                            for hl in range(2)]
                    for t in range(4 * j + 4):
                        tl = t - 4 * j
                        o0 = 128 * tl if tl > 0 else 0   # skip fully-masked cols
                        for hl in range(2):
                            hp = 64 * hl
                            pss = ps_s.tile([128, CH], F32, tag="s", name="pss")
                            nc.tensor.matmul(
                                pss[:, o0:CH],
                                kT_sb[hp:hp + 64, b, 128 * t:128 * t + 128],
                                qT_sb[hp:hp + 64, b, ls + o0:ls + CH],
                                start=True, stop=True)
                            pt = ptpool.tile([128, CH], BF16, name="pt")
                            nc.scalar.activation(pt[:, o0:CH], pss[:, o0:CH],
                                                 AF.Exp, scale=0.125)
                            if tl >= 0:
                                nc.vector.tensor_mul(pt[:, o0:o0 + 128],
                                                     pt[:, o0:o0 + 128],
                                                     tri2_sb[:, 0])
                            nc.tensor.matmul(po_t[hl][:, o0:CH],
                                             v_sb[:, b, t, 65 * hl:65 * hl + 65],
                                             pt[:, o0:CH], start=(t == 0),
                                             stop=(t == 4 * j + 3),
                                             skip_group_check=True)

                    # ---- epilogue: divide by denominator, store to send buffer ----
                    o_sb = opool.tile([128, CH], BF16, name="o_sb")
                    for hl in range(2):
                        # NB: custom-DVE recip misreads PSUM at base partition 64,
                        # so stage the denominator row through SBUF first.
                        dr = tpool.tile([1, CH], F32, name="dr", bufs=2)
                        nc.vector.tensor_copy(dr[:], po_t[hl][64:65, :])
                        den = tpool.tile([1, CH], F32, name="den", bufs=2)
                        nc.vector.reciprocal_approx_fast(den[:], dr[:])
                        rb = tpool.tile([64, CH], F32, name="rb", bufs=2)
                        nc.gpsimd.partition_broadcast(rb[:], den[:])
                        nc.vector.tensor_mul(o_sb[64 * hl:64 * hl + 64, :],
                                             po_t[hl][0:64, :], rb[:])
                    nc.sync.dma_start(ag_in[j][128 * b:128 * b + 128, :], o_sb[:])
                    if b == B - 1:
                        nc.gpsimd.collective_compute(
                            "AllGather", ALU.bypass,
                            replica_groups=[list(range(NC))],
                            ins=[ag_in[j][:].opt()],
                            outs=[ag_all[j].opt()],
                        )

            # ---- output projection on this core's own gathered block ----
            pid = nc.gpsimd.partition_id()
            bidx = pid // NCH
            jidx = pid % NCH
            ag_view = ag_all.rearrange("blk (s b p) l -> blk b p s l", b=B, p=128)
            oF = fpool.tile([128, 8, CH], BF16, bufs=1)
            for kt in range(8):
                nc.gpsimd.dma_start(
                    oF[:, kt],
                    ag_view[bass.ds(jidx, 1), bass.ds(bidx, 1), :, kt].opt())
            for mt in range(4):
                for nn in range(2):
                    py_ = ps_proj.tile([128, 512], F32, tag="proj", name="py")
                    for kt in range(8):
                        nc.tensor.matmul(py_[:], oF[:, kt, 128 * mt:128 * mt + 128],
                                         wo_sb[:, kt, 512 * nn:512 * nn + 512],
                                         start=(kt == 0), stop=(kt == 7))
                    ysb = fpool.tile([128, 512], F32, name="ysb")
                    nc.vector.tensor_copy(ysb[:], py_[:])
                    nc.sync.dma_start(out_e[128 * mt:128 * mt + 128, 512 * nn:512 * nn + 512],
                                      ysb[:])

    nc.compile()
    return nc


def _shards(x, Wq, Wk, Wv, Wo, cos, sin):
    bf = ml_dtypes.bfloat16
    xT = np.ascontiguousarray(x.transpose(0, 2, 1)).astype(bf)          # (B, HID, L)
    woT = np.ascontiguousarray(Wo.T).astype(bf)                          # (HID, HID)
    cosT = cos.T.astype(np.float32)                                      # (D, L)
    sinT = sin.T.astype(np.float32)
    cos2 = np.concatenate([cosT, cosT], axis=0)                          # (128, L)
    spm = np.concatenate([-sinT[:32], sinT[32:]], axis=0)                # (64, L)
    sinpm = np.ascontiguousarray(np.concatenate([spm, spm], axis=0))     # (128, L)
    # lower-triangular 128x128 mask for the diagonal band
    p = np.arange(128)[:, None]
    f = np.arange(128)[None, :]
    tri = (p <= f).astype(np.float32).astype(bf)                         # (128, 128)

    in_maps = []
    for c in range(NC):
        rows = slice(2 * c * 64, (2 * c + 2) * 64)
        in_maps.append({
            "xT": xT,
            "wq": np.ascontiguousarray(Wq[rows].T).astype(bf),
            "wk": np.ascontiguousarray(Wk[rows].T).astype(bf),
            "wv": np.ascontiguousarray(Wv[rows].T).astype(bf),
            "wo": woT,
            "cos2": cos2,
            "sinpm": sinpm,
            "tri": tri,
        })
    return in_maps


def kernel(x, Wq, Wk, Wv, Wo, cos, sin, trace=False):
    x = np.asarray(x, dtype=np.float32)
    Wq = np.asarray(Wq, dtype=np.float32)
    Wk = np.asarray(Wk, dtype=np.float32)
    Wv = np.asarray(Wv, dtype=np.float32)
    Wo = np.asarray(Wo, dtype=np.float32)
    cos = np.asarray(cos, dtype=np.float32)
    sin = np.asarray(sin, dtype=np.float32)

    if "nc" not in _CACHE:
        _CACHE["nc"] = build()
    nc = _CACHE["nc"]

    in_maps = _shards(x, Wq, Wk, Wv, Wo, cos, sin)
    res = bass_utils.run_bass_kernel_spmd(
        nc, in_maps, core_ids=list(range(NC)), trace=trace)
    _CACHE["last_result"] = res

    y = np.empty((B, L, HID), dtype=np.float32)
    for c in range(NC):
        b, blk = c // 4, c % 4
        y[b, blk * CH:(blk + 1) * CH, :] = res.results[c]["out"]
    return y


if __name__ == "__main__":
    rng = np.random.default_rng(0)
    sc = 1.0 / np.sqrt(HID)
    inputs = {
        "x": rng.standard_normal((B, L, HID), dtype=np.float32),
        "Wq": rng.standard_normal((HID, HID), dtype=np.float32) * sc,
        "Wk": rng.standard_normal((HID, HID), dtype=np.float32) * sc,
        "Wv": rng.standard_normal((HID, HID), dtype=np.float32) * sc,
        "Wo": rng.standard_normal((HID, HID), dtype=np.float32) * sc,
        "cos": rng.random((L, D), dtype=np.float32),
        "sin": rng.random((L, D), dtype=np.float32),
    }
    y = kernel(**inputs)
    print("ran:", y.shape, y.dtype)


# revision 33
# speedup vs baseline: 1.3503x; 1.3503x over previous
"""Distributed RoPE causal attention for Trainium2 (8 NeuronCores).

Problem: B=2, L=2048, H=16 heads, D=64; y = Attn(x) with RoPE on q,k,
causal softmax, then output projection. fp32 I/O.

Sharding: each core owns 2 heads (tensor-parallel over the 16 heads) for
BOTH batches; after attention, a single 8-core AllToAll redistributes the
per-head outputs so each core holds the full hidden vector for one
(batch, L/4-block) slice, on which it runs the (replicated-weight) output
projection. Matmuls run in bf16 (fp32 PSUM accumulation); RoPE/softmax
plumbing in fp32.
"""
import sys

sys.path.insert(0, "/opt/trn_rl_repo")

import numpy as np
import ml_dtypes

import concourse.bass as bass
import concourse.tile as tile
from concourse import bacc, mybir
from concourse import bass_utils

B, L, H, D = 2, 2048, 16, 64
HID = H * D
NC = 8
CH = 512          # lq chunk width
NCH = L // CH     # 4 chunks per batch
NT = L // 128     # 16 k-tiles of 128 per batch
F32 = mybir.dt.float32
BF16 = mybir.dt.bfloat16
AF = mybir.ActivationFunctionType
ALU = mybir.AluOpType

_CACHE = {}


def build():
    nc = bacc.Bacc("TRN2", target_bir_lowering=False, debug=False, num_devices=NC)

    xT_e = nc.dram_tensor("xT", [B, HID, L], BF16, kind="ExternalInput")
    wq_e = nc.dram_tensor("wq", [HID, 128], BF16, kind="ExternalInput")
    wk_e = nc.dram_tensor("wk", [HID, 128], BF16, kind="ExternalInput")
    wv_e = nc.dram_tensor("wv", [HID, 128], BF16, kind="ExternalInput")
    wo_e = nc.dram_tensor("wo", [HID, HID], BF16, kind="ExternalInput")
    cos_e = nc.dram_tensor("cos2", [128, L], F32, kind="ExternalInput")
    sin_e = nc.dram_tensor("sinpm", [128, L], F32, kind="ExternalInput")
    tri_e = nc.dram_tensor("tri", [128, 128], BF16, kind="ExternalInput")
    out_e = nc.dram_tensor("out", [CH, HID], F32, kind="ExternalOutput")

    xT_r = xT_e.ap().rearrange("b (kt p) l -> b p kt l", p=128)

    with tile.TileContext(nc) as tc:
        with tc.tile_pool(name="const", bufs=1) as cpool, \
             tc.tile_pool(name="persist", bufs=1) as ppool, \
             tc.tile_pool(name="xin", bufs=3) as xpool, \
             tc.tile_pool(name="tmp", bufs=3) as tpool, \
             tc.tile_pool(name="ptp", bufs=6) as ptpool, \
             tc.tile_pool(name="osb", bufs=2) as opool, \
             tc.tile_pool(name="fin", bufs=2) as fpool, \
             tc.tile_pool(name="ps_proj", bufs=2, space="PSUM") as ps_proj, \
             tc.tile_pool(name="ps_s", bufs=2, space="PSUM") as ps_s, \
             tc.tile_pool(name="ps_o", bufs=2, space="PSUM") as ps_o, \
             tc.tile_pool(name="dram", bufs=1, space="DRAM") as dpool:

            wq_sb = cpool.tile([128, 8, 128], BF16)
            wk_sb = cpool.tile([128, 8, 128], BF16)
            wv_sb = cpool.tile([128, 8, 128], BF16)
            wo_sb = cpool.tile([128, 8, HID], BF16)
            nc.sync.dma_start(wq_sb[:], wq_e.ap().rearrange("(kt p) m -> p kt m", p=128))
            nc.sync.dma_start(wk_sb[:], wk_e.ap().rearrange("(kt p) m -> p kt m", p=128))
            nc.sync.dma_start(wv_sb[:], wv_e.ap().rearrange("(kt p) m -> p kt m", p=128))
            cos_sb = cpool.tile([128, L], F32)
            sin_sb = cpool.tile([128, L], F32)
            tri2_sb = cpool.tile([128, 2, 128], BF16)
            nc.sync.dma_start(tri2_sb[:, 0], tri_e[:, :])
            nc.sync.dma_start(tri2_sb[:, 1], tri_e[:, :])

            # persistent per-core tensors
            qT_sb = ppool.tile([128, B, L], BF16)   # [64*hl+d, b, l]
            kT_sb = ppool.tile([128, B, L], BF16)
            v_sb = ppool.tile([128, B, NT, 130], BF16)  # [lk%128, b, lk//128, 65*hl + (d|ones)]

            # one AllGather per lq-block (both batches), pipelined behind compute;
            # each core later picks its own (batch, block) out of ag_all by core id
            ag_all = dpool.tile([NCH, NC * 256, CH], BF16)
            ag_in = [dpool.tile([256, CH], BF16, name=f"agin{i}", tag=f"agin{i}")
                     for i in range(NCH)]

            for j in range(NCH):
                for b in range(B):
                    if j == 1 and b == 0:
                        # late-load the (big) output-projection weight: only needed
                        # after the AllGathers, keeps startup DMA on the critical inputs
                        nc.sync.dma_start(
                            wo_sb[:], wo_e.ap().rearrange("(kt p) m -> p kt m", p=128))
                    ls = j * CH
                    xc = xpool.tile([128, 8, CH], BF16, name="xc")
                    nc.sync.dma_start(xc[:], xT_r[b][:, :, ls:ls + CH])
                    if b == 0:
                        nc.sync.dma_start(cos_sb[:, ls:ls + CH], cos_e[:, ls:ls + CH])
                        nc.sync.dma_start(sin_sb[:, ls:ls + CH], sin_e[:, ls:ls + CH])

                    # ---- Q, K projections + RoPE ----
                    for w_sb, dst in ((wq_sb, qT_sb), (wk_sb, kT_sb)):
                        pp = ps_proj.tile([128, CH], F32, tag="proj", name="pp")
                        for kt in range(8):
                            nc.tensor.matmul(pp[:], w_sb[:, kt], xc[:, kt],
                                             start=(kt == 0), stop=(kt == 7))
                        t1 = tpool.tile([128, CH], F32, name="t1")
                        nc.vector.tensor_mul(t1[:], pp[:], cos_sb[:, ls:ls + CH])
                        t2 = tpool.tile([128, CH], F32, name="t2")
                        for seg in range(4):
                            po, pi = 32 * seg, 32 * (seg ^ 1)
                            nc.vector.tensor_mul(t2[po:po + 32, :], pp[pi:pi + 32, :],
                                                 sin_sb[po:po + 32, ls:ls + CH])
                        nc.vector.tensor_add(dst[:, b, ls:ls + CH], t1[:], t2[:])

                    # ---- V projection (+ ones column for denominators) ----
                    for tl in range(4):
                        t = 4 * j + tl
                        pv = ps_proj.tile([128, 128], F32, tag="proj", name="pv")
                        for kt in range(8):
                            nc.tensor.matmul(pv[:], xc[:, kt, 128 * tl:128 * tl + 128],
                                             wv_sb[:, kt], start=(kt == 0), stop=(kt == 7))
                        nc.vector.memset(v_sb[:, b, t, 64::65], 1.0)
                        vv = v_sb.rearrange("p b t (hl e) -> p b t hl e", e=65)
                        nc.vector.tensor_copy(
                            vv[:, b, t, :, 0:64],
                            pv.rearrange("p (hl d) -> p hl d", d=64),
                        )

                    # ---- attention for lq chunk j, both local heads ----
                    po_t = [ps_o.tile([65, CH], F32, tag="o", name=f"po{hl}")
                            for hl in range(2)]
                    for t in range(4 * j + 4):
                        tl = t - 4 * j
                        o0 = 128 * tl if tl > 0 else 0   # skip fully-masked cols
                        # both heads' scores go into one 2-bank psum tile so
                        # exp and the diagonal mask run as single wider ops
                        pss = ps_s.tile([128, 2, CH], F32, tag="s", name="pss")
                        for hl in range(2):
                            hp = 64 * hl
                            nc.tensor.matmul(
                                pss[:, hl, o0:CH],
                                kT_sb[hp:hp + 64, b, 128 * t:128 * t + 128],
                                qT_sb[hp:hp + 64, b, ls + o0:ls + CH],
                                start=True, stop=True)
                        pt = ptpool.tile([128, 2, CH], BF16, name="pt")
                        nc.scalar.activation(pt[:, :, o0:CH], pss[:, :, o0:CH],
                                             AF.Exp, scale=0.125)
                        if tl >= 0:
                            nc.vector.tensor_mul(pt[:, :, o0:o0 + 128],
                                                 pt[:, :, o0:o0 + 128], tri2_sb[:])
                        for hl in range(2):
                            nc.tensor.matmul(po_t[hl][:, o0:CH],
                                             v_sb[:, b, t, 65 * hl:65 * hl + 65],
                                             pt[:, hl, o0:CH], start=(t == 0),
                                             stop=(t == 4 * j + 3),
                                             skip_group_check=True)

                    # ---- epilogue: divide by denominator, store to send buffer ----
                    o_sb = opool.tile([128, CH], BF16, name="o_sb")
                    for hl in range(2):
                        # NB: custom-DVE recip misreads PSUM at base partition 64,
                        # so stage the denominator row through SBUF first.
                        dr = tpool.tile([1, CH], F32, name="dr", bufs=2)
                        nc.vector.tensor_copy(dr[:], po_t[hl][64:65, :])
                        den = tpool.tile([1, CH], F32, name="den", bufs=2)
                        nc.vector.reciprocal_approx_fast(den[:], dr[:])
                        rb = tpool.tile([64, CH], F32, name="rb", bufs=2)
                        nc.gpsimd.partition_broadcast(rb[:], den[:])
                        nc.vector.tensor_mul(o_sb[64 * hl:64 * hl + 64, :],
                                             po_t[hl][0:64, :], rb[:])
                    nc.sync.dma_start(ag_in[j][128 * b:128 * b + 128, :], o_sb[:])
                    if b == B - 1:
                        nc.gpsimd.collective_compute(
                            "AllGather", ALU.bypass,
                            replica_groups=[list(range(NC))],
                            ins=[ag_in[j][:].opt()],
                            outs=[ag_all[j].opt()],
                        )

            # ---- output projection on this core's own gathered block ----
            pid = nc.gpsimd.partition_id()
            bidx = pid // NCH
            jidx = pid % NCH
            ag_view = ag_all.rearrange("blk (s b p) l -> blk b p s l", b=B, p=128)
            oF = fpool.tile([128, 8, CH], BF16, bufs=1)
            for kt in range(8):
                nc.gpsimd.dma_start(
                    oF[:, kt],
                    ag_view[bass.ds(jidx, 1), bass.ds(bidx, 1), :, kt].opt())
            for mt in range(4):
                for nn in range(2):
                    py_ = ps_proj.tile([128, 512], F32, tag="proj", name="py")
                    for kt in range(8):
                        nc.tensor.matmul(py_[:], oF[:, kt, 128 * mt:128 * mt + 128],
                                         wo_sb[:, kt, 512 * nn:512 * nn + 512],
                                         start=(kt == 0), stop=(kt == 7))
                    ysb = fpool.tile([128, 512], F32, name="ysb")
                    nc.vector.tensor_copy(ysb[:], py_[:])
                    nc.sync.dma_start(out_e[128 * mt:128 * mt + 128, 512 * nn:512 * nn + 512],
                                      ysb[:])

    nc.compile()
    return nc


def _shards(x, Wq, Wk, Wv, Wo, cos, sin):
    bf = ml_dtypes.bfloat16
    xT = np.ascontiguousarray(x.transpose(0, 2, 1)).astype(bf)          # (B, HID, L)
    woT = np.ascontiguousarray(Wo.T).astype(bf)                          # (HID, HID)
    cosT = cos.T.astype(np.float32)                                      # (D, L)
    sinT = sin.T.astype(np.float32)
    cos2 = np.concatenate([cosT, cosT], axis=0)                          # (128, L)
    spm = np.concatenate([-sinT[:32], sinT[32:]], axis=0)                # (64, L)
    sinpm = np.ascontiguousarray(np.concatenate([spm, spm], axis=0))     # (128, L)
    # lower-triangular 128x128 mask for the diagonal band
    p = np.arange(128)[:, None]
    f = np.arange(128)[None, :]
    tri = (p <= f).astype(np.float32).astype(bf)                         # (128, 128)

    in_maps = []
    for c in range(NC):
        rows = slice(2 * c * 64, (2 * c + 2) * 64)
        in_maps.append({
            "xT": xT,
            "wq": np.ascontiguousarray(Wq[rows].T).astype(bf),
            "wk": np.ascontiguousarray(Wk[rows].T).astype(bf),
            "wv": np.ascontiguousarray(Wv[rows].T).astype(bf),
            "wo": woT,
            "cos2": cos2,
            "sinpm": sinpm,
            "tri": tri,
        })
    return in_maps


def kernel(x, Wq, Wk, Wv, Wo, cos, sin, trace=False):
    x = np.asarray(x, dtype=np.float32)
    Wq = np.asarray(Wq, dtype=np.float32)
    Wk = np.asarray(Wk, dtype=np.float32)
    Wv = np.asarray(Wv, dtype=np.float32)
    Wo = np.asarray(Wo, dtype=np.float32)
    cos = np.asarray(cos, dtype=np.float32)
    sin = np.asarray(sin, dtype=np.float32)

    if "nc" not in _CACHE:
        _CACHE["nc"] = build()
    nc = _CACHE["nc"]

    in_maps = _shards(x, Wq, Wk, Wv, Wo, cos, sin)
    res = bass_utils.run_bass_kernel_spmd(
        nc, in_maps, core_ids=list(range(NC)), trace=trace)
    _CACHE["last_result"] = res

    y = np.empty((B, L, HID), dtype=np.float32)
    for c in range(NC):
        b, blk = c // 4, c % 4
        y[b, blk * CH:(blk + 1) * CH, :] = res.results[c]["out"]
    return y


if __name__ == "__main__":
    rng = np.random.default_rng(0)
    sc = 1.0 / np.sqrt(HID)
    inputs = {
        "x": rng.standard_normal((B, L, HID), dtype=np.float32),
        "Wq": rng.standard_normal((HID, HID), dtype=np.float32) * sc,
        "Wk": rng.standard_normal((HID, HID), dtype=np.float32) * sc,
        "Wv": rng.standard_normal((HID, HID), dtype=np.float32) * sc,
        "Wo": rng.standard_normal((HID, HID), dtype=np.float32) * sc,
        "cos": rng.random((L, D), dtype=np.float32),
        "sin": rng.random((L, D), dtype=np.float32),
    }
    y = kernel(**inputs)
    print("ran:", y.shape, y.dtype)
